# revision 7
# baseline (speedup 1.0000x reference)
"""CavemanGPT single-head attention on 8 Trainium2 NeuronCores.

Math (reference; its mask input is unused there):
    Q = emb @ W_q^T ; K = emb @ W_k^T ; V = emb @ W_v^T        (per batch b)
    out = softmax(K @ Q^T / sqrt(H), axis=-1) @ V

Algebraic restructure 1 (from the baseline): K @ Q^T = emb @ (W_k^T W_q) @ emb^T,
so with G := W_k^T @ W_q ([E, E], batch independent) the per-core work drops
~3.2x and the giant [S, H] Q/K intermediates vanish.

Restructure 2 (new): W_q/W_k are uniform[0,1) = 0.5 + centered part, so
    G = 0.25*H*1 1^T + 0.5*1 a^T + 0.5*b 1^T + Ghat,   Ghat = Wkc^T @ Wqc
with a = Wqc^T 1, b = Wkc^T 1.  The rank-2 part carries ~99% of the score
magnitude (|scores| to 1.1e7) and is computed EXACTLY via an 8-row fp16
matmul from host-precomputed vectors (s_i = sum(emb_i), c_j = (H/4)s_j +
p_j/2, q_i = b.emb_i):
    scores = s_i c_j + q_i s_j/2 + emb_i Ghat emb_j^T
The residual Ghat-scores are ~180x smaller (std ~8e3), so the fp32-grade
hi/lo-limb chain of the baseline is overkill for them.  Each residual
product x*y with fp16 limbs (xh+xl)(yh+yl) is computed as ONE fp16 matmul
(xh*yh) plus ONE fp8e4 DoubleRow matmul computing BOTH cross terms
(xh*yl + xl*yh) in a single pass at 2x rate -- ~2/3 the PE time of the
baseline's 3x fp16 limb scheme with BETTER accuracy (the cross terms only
need ~5 bits below the hi product; e4m3 quantization of the limbs leaves
~2^-15.5-grade products, vs needing ~14.5 bits for the residual).

Softmax here is a hard argmax (top-2 score gaps median ~870 in exp-arg
units, min 0.21): correctness = never flip an argmax; verified exactly in
simulation on the real inputs (0 flips, max rel err 8.1e-4, identical to
the full 3-limb fp16 scheme -- the fp16 attn/V floor).

Two launches:
  1. G-launch: Ghat = Wkc^T @ Wqc sharded over 8 cores (2 e'-halves x 4
     h-quarters), fp16 hh + fp8 DoubleRow cross; host sums the h-partials.
     Plus one (batch, j-half) shard of V = emb @ W_v^T per core (fp16).
  2. Main launch: 8 cores = 4 batches x 2 halves of the i (output-row)
     dimension, each producing out[i-half] via residual scores + rank-2
     matmul + softmax + attn @ V.
"""

import math

import numpy as np
import ml_dtypes

import concourse.bass as bass
import concourse.bass_utils as _bu
import concourse.mybir as mybir
import concourse.tile as tile
from concourse import bacc
from concourse.bass_utils import run_bass_kernel_spmd
from concourse.masks import make_identity

# LDWEIGHTS dedup: consecutive matmuls sharing a stationary operand skip the
# reload. Verified to produce bit-identical output on this kernel.
if not getattr(_bu, "_ldw_opt_patched", False):
    _orig_walrus_args = _bu.get_walrus_args

    def _walrus_args_ldw(arch, tmpdir, *, dve_root=None):
        args = _orig_walrus_args(arch, tmpdir, dve_root=dve_root)
        return [a.replace("--enable-ldw-opt=false", "--enable-ldw-opt=true") for a in args]

    _bu.get_walrus_args = _walrus_args_ldw
    _bu._ldw_opt_patched = True

dt = mybir.dt
F8 = ml_dtypes.float8_e4m3
P = 128
N_CORES = 8
DR = mybir.MatmulPerfMode.DoubleRow


def _split16(x):
    """x (fp32) -> (hi, lo) fp16 limbs with x ~= hi + lo (22-bit mantissa)."""
    x = np.ascontiguousarray(x, dtype=np.float32)
    hi = x.astype(np.float16)
    lo = (x - hi.astype(np.float32)).astype(np.float16)
    return hi, lo


def _q8(x, scale):
    """fp32 -> TRN e4m3 of x*scale (saturating clip to +-240)."""
    y = np.clip(np.asarray(x, np.float32) * scale, -240.0, 240.0)
    return np.ascontiguousarray(y).astype(F8)


def build_g_nc(S, E, H, O):
    """Launch 1: per-core partial Ghat' = Wkc[hq]^T @ Wqc[hq][:, e'half]
    (PSUM = (32Wkc)^T(32Wqc) = 1024*Ghat) plus one (batch, j-half) shard of
    V = embT^T @ WvT (single fp16).

    Limb scheme per 128-h chunk: 1 fp16 matmul (Kh^T Qh) + 1 fp8e4 DoubleRow
    matmul computing Kh^T Ql + Kl^T Qh (stored pre-scaled so both products
    land in the same PSUM units).
    """
    SI = S // 2
    EH = E // 2
    HQ = H // 4
    EB = E // P
    HCB = HQ // P
    JBH = SI // P
    GW = min(512, EH)
    OW = min(512, O)
    NOW = O // OW
    f32, f16, f8 = dt.float32, dt.float16, dt.float8e4

    nc = bacc.Bacc("TRN2", target_bir_lowering=False, debug=False)
    kh16 = nc.dram_tensor("kh16", [HQ, E], f16, kind="ExternalInput").ap()
    k8 = nc.dram_tensor("k8", [HQ, 2, E], f8, kind="ExternalInput").ap()
    qh16 = nc.dram_tensor("qh16", [HQ, EH], f16, kind="ExternalInput").ap()
    q8 = nc.dram_tensor("q8", [HQ, 2, EH], f8, kind="ExternalInput").ap()
    evt = nc.dram_tensor("evt", [E, SI], f16, kind="ExternalInput").ap()
    wvt = nc.dram_tensor("wvt", [E, O], f16, kind="ExternalInput").ap()
    g_part = nc.dram_tensor("g_part", [E, EH], f32, kind="ExternalOutput").ap()
    v_part = nc.dram_tensor("v_part", [SI, O], f16, kind="ExternalOutput").ap()

    with tile.TileContext(nc) as tc:
        with (
            tc.tile_pool(name="p_res", bufs=1) as p_res,
            tc.tile_pool(name="p_vo", bufs=2) as p_vo,
            tc.tile_pool(name="p_gs", bufs=3) as p_gs,
            tc.tile_pool(name="ps_g", bufs=8, space="PSUM") as ps_g,
        ):
            # ---- PE warm-up: ~3.5us of dummy matmuls during the DMA
            # preamble trips the HAM clock-gate so real matmuls start at
            # 2.4GHz instead of 1.2 ----
            wu = p_res.tile([P, P], f16)
            nc.gpsimd.memset(wu[:], 0.0)
            wups = ps_g.tile([P, P], f32, tag="gps", name="wups")
            for _ in range(48):
                nc.tensor.matmul(wups[:], wu[:], wu[:], start=True, stop=True)

            # ---- Ghat partial ----
            gp = p_res.tile([P, EB, EH], f32)
            evc = p_res.tile([P, EB, SI], f16)
            wvc = p_res.tile([P, EB, O], f16)
            evr = evt.rearrange("(eo p) j -> p eo j", p=P)
            wvr = wvt.rearrange("(eo p) o -> p eo o", p=P)
            pt_g = [
                ps_g.tile([P, GW], f32, tag="gps", name=f"gps_{eb}")
                for eb in range(EB)
            ]
            for hc in range(HCB):
                hs = slice(hc * P, (hc + 1) * P)
                kh = p_gs.tile([P, E], f16, tag="kh")
                nc.sync.dma_start(kh[:], kh16[hs, :])
                qh = p_gs.tile([P, EH], f16, tag="qh")
                nc.sync.dma_start(qh[:], qh16[hs, :])
                k8t = p_gs.tile([P, 2, E], f8, tag="k8")
                nc.sync.dma_start(k8t[:], k8[hs])
                q8t = p_gs.tile([P, 2, EH], f8, tag="q8")
                nc.sync.dma_start(q8t[:], q8[hs])
                # stream the V inputs alongside the k/q chunks so the V
                # matmuls can start the moment the Ghat ones finish
                nc.sync.dma_start(evc[:, hc], evr[:, hc])
                nc.sync.dma_start(wvc[:, hc], wvr[:, hc])
                first, last = hc == 0, hc == HCB - 1
                for eb in range(EB):
                    ksl = slice(eb * P, (eb + 1) * P)
                    nc.tensor.matmul(
                        pt_g[eb][:], kh[:, ksl], qh[:], start=first, stop=False,
                    )
                    nc.tensor.matmul(
                        pt_g[eb][:], k8t[:, :, ksl], q8t[:],
                        start=False, stop=last, perf_mode=DR,
                    )
            gpr = g_part.rearrange("(eo p) e2 -> p eo e2", p=P)
            for eb in range(EB):
                nc.vector.tensor_scalar_mul(gp[:, eb], pt_g[eb][:], 2.0**-10)
                # overlap the writeback with the remaining evacuations
                nc.sync.dma_start(gpr[:, eb], gp[:, eb])

            # ---- V shard (PE runs it after Ghat; inputs loaded during it) ----
            for jb in range(JBH):
                jsl = slice(jb * P, (jb + 1) * P)
                pv_tiles = [
                    ps_g.tile([P, OW], f32, tag="gps", name=f"vps_{jb}_{ob}")
                    for ob in range(NOW)
                ]
                for eb in range(EB):
                    for ob in range(NOW):
                        osl = slice(ob * OW, (ob + 1) * OW)
                        nc.tensor.matmul(
                            pv_tiles[ob][:], evc[:, eb, jsl], wvc[:, eb, osl],
                            start=(eb == 0), stop=(eb == EB - 1),
                        )
                vt = p_vo.tile([P, O], f16, tag="vt")
                for ob in range(NOW):
                    osl = slice(ob * OW, (ob + 1) * OW)
                    nc.vector.tensor_scalar_mul(vt[:, osl], pv_tiles[ob][:], 2.0**-5)
                    nc.sync.dma_start(v_part[jsl, osl], vt[:, osl])

    nc.compile()
    return nc


def build_main_nc(S, E, H, O):
    """Launch 2: attention for one (batch, i-half).

    Residual chain (PSUM units: stage1 = 128*Ghat@emb, stage2 = raw/2):
      AT = Ghat^T @ embT:  Gh16*Eh (fp16) + DoubleRow[gh8*el8 + gl8*eh8]
      scores: Ah*Eh (fp16) + DoubleRow[ah8*el8 + al8*eh8] + 8-row rank matmul
    Stored forms: Eh = fp16(32 emb^T), e8 = [e4m3(4 El) | e4m3(Eh/32)],
    Gh/Gl = fp16 pair of 4*Ghat, g8 = [e4m3(Gh/4) | e4m3(32 Gl)],
    Ah = fp16(AT*2^-6) (psum*2^-13), a8 = [e4m3(Ah/4) | e4m3(32 Al)].
    Rank matmul rows (i-side | j-side): fp16 limb pairs of
    (2s_i, q_i/8 | c_j/4, 2s_j) arranged so their PSUM sum is R_ij/2.
    """
    SI = S // 2          # i rows per core
    EB = E // P          # 128-chunks of the embedding dim
    JB = S // P
    IB = SI // P
    IW = min(512, SI)    # AT moving width along i
    NIH = SI // IW
    JW = min(512, S)     # scores moving width along j
    NJW = S // JW
    OW = min(512, O)
    NOW = O // OW
    SCALE_EXP = 2.0 / math.sqrt(H)   # PSUM = raw/2

    f32, f16, f8 = dt.float32, dt.float16, dt.float8e4

    nc = bacc.Bacc("TRN2", target_bir_lowering=False, debug=False)
    g_h = nc.dram_tensor("g_h", [E, E], f16, kind="ExternalInput").ap()
    g_8 = nc.dram_tensor("g_8", [E, 2, E], f8, kind="ExternalInput").ap()
    et_h = nc.dram_tensor("et_h", [E, S], f16, kind="ExternalInput").ap()
    et_8 = nc.dram_tensor("et_8", [E, 2, S], f8, kind="ExternalInput").ap()
    v_in = nc.dram_tensor("v_in", [S, O], f16, kind="ExternalInput").ap()
    rk_i = nc.dram_tensor("rk_i", [8, SI], f16, kind="ExternalInput").ap()
    rk_j = nc.dram_tensor("rk_j", [8, S], f16, kind="ExternalInput").ap()
    out = nc.dram_tensor("out", [SI, O], f32, kind="ExternalOutput").ap()

    with tile.TileContext(nc) as tc:
        with (
            tc.tile_pool(name="misc", bufs=2) as misc,
            tc.tile_pool(name="p_big", bufs=1) as p_big,
        ):
            ident = misc.tile([P, P], f16, tag="ident", name="ident")
            make_identity(nc, ident[:])
            wu = misc.tile([P, P], f16, tag="wu", name="wu")
            nc.gpsimd.memset(wu[:], 0.0)

            # whole-kernel residents
            eth = p_big.tile([P, EB, S], f16)     # Eh: [e part, e chunk, tok]
            e8 = p_big.tile([P, EB, 2, S], f8)    # [4El | Eh/32]
            ath = p_big.tile([P, EB, SI], f16)    # Ah: [e' part, e' chunk, i]
            a8 = p_big.tile([P, EB, 2, SI], f8)   # [Ah/4 | 32Al]
            v16 = p_big.tile([P, JB, O], f16)     # V: [j part, j chunk, o]
            rki = p_big.tile([8, SI], f16)
            rkj = p_big.tile([8, S], f16)

            with tc.tile_pool(name="ps", bufs=8, space="PSUM") as ps:
                # PE warm-up during the input-DMA preamble (see launch 1)
                wups = ps.tile([P, P], f32, tag="ps", name="wups")
                for _ in range(48):
                    nc.tensor.matmul(wups[:], wu[:], wu[:], start=True, stop=True)

                # ---- AT = Ghat^T embT (PSUM = 128*Ghat@emb -> *2^-13) ----
                with tc.tile_pool(name="p_g", bufs=1) as p_g:
                    gh = p_g.tile([P, EB, E], f16)
                    g8t = p_g.tile([P, EB, 2, E], f8)
                    # DMAs emitted in first-use order, chunked per e-block so
                    # the first AT matmuls start early.
                    ghr = g_h.rearrange("(eo p) e2 -> p eo e2", p=P)
                    g8r = g_8.rearrange("(eo p) two e2 -> p eo two e2", p=P)
                    ethr = et_h.rearrange("(eo p) t -> p eo t", p=P)
                    e8r = et_8.rearrange("(eo p) two t -> p eo two t", p=P)
                    for eb in range(EB):
                        nc.sync.dma_start(gh[:, eb], ghr[:, eb])
                        nc.sync.dma_start(eth[:, eb, :SI], ethr[:, eb, :SI])
                        nc.sync.dma_start(g8t[:, eb], g8r[:, eb])
                        nc.sync.dma_start(e8[:, eb, :, :SI], e8r[:, eb, :, :SI])
                    if SI < S:
                        nc.sync.dma_start(eth[:, :, SI:], ethr[:, :, SI:])
                        for eb in range(EB):
                            nc.sync.dma_start(
                                e8[:, eb, :, SI:], e8r[:, eb, :, SI:]
                            )
                    nc.sync.dma_start(rkj[:], rk_j)
                    nc.sync.dma_start(rki[:], rk_i)
                    nc.sync.dma_start(
                        v16[:], v_in.rearrange("(jo p) o -> p jo o", p=P)
                    )
                    for ih in range(NIH):
                        isl = slice(ih * IW, (ih + 1) * IW)
                        pts = [
                            ps.tile([P, IW], f32, tag="ps", name=f"aps_{ih}_{epb}")
                            for epb in range(EB)
                        ]
                        for eb in range(EB):
                            first, last = eb == 0, eb == EB - 1
                            for epb in range(EB):
                                psl = slice(epb * P, (epb + 1) * P)
                                nc.tensor.matmul(
                                    pts[epb][:], gh[:, eb, psl], eth[:, eb, isl],
                                    start=first, stop=False,
                                )
                                nc.tensor.matmul(
                                    pts[epb][:], g8t[:, eb, :, psl],
                                    e8[:, eb, :, isl],
                                    start=False, stop=last, perf_mode=DR,
                                )
                        for epb in range(EB):
                            pt = pts[epb]
                            atmp = misc.tile([P, IW], f32, tag="atmp", name=f"atmp_{ih}_{epb}")
                            alo = misc.tile([P, IW], f32, tag="alo", name=f"alo_{ih}_{epb}")
                            nc.vector.tensor_scalar_mul(atmp[:], pt[:], 2.0**-13)
                            nc.vector.tensor_copy(ath[:, epb, isl], atmp[:])
                            nc.vector.tensor_tensor(
                                alo[:], atmp[:], ath[:, epb, isl],
                                mybir.AluOpType.subtract,
                            )
                            # fp8 limb casts on the (idle) scalar engine
                            nc.scalar.activation(
                                a8[:, epb, 0, isl], ath[:, epb, isl],
                                mybir.ActivationFunctionType.Copy, scale=0.25,
                            )
                            nc.scalar.activation(
                                a8[:, epb, 1, isl], alo[:],
                                mybir.ActivationFunctionType.Copy, scale=32.0,
                            )

                # ---- scores + softmax + out, fused per 128-row i block ----
                with (
                    tc.tile_pool(name="p_sw", bufs=2) as p_sw,
                    tc.tile_pool(name="p_sw1", bufs=2) as p_sw1,
                ):
                    def emit_scores(ib):
                        ibs = slice(ib * P, (ib + 1) * P)
                        pt_s = [
                            ps.tile([P, JW], f32, tag="ps", name=f"sps_{ib}_{w}")
                            for w in range(NJW)
                        ]
                        for epb in range(EB):
                            for w in range(NJW):
                                wsl = slice(w * JW, (w + 1) * JW)
                                nc.tensor.matmul(
                                    pt_s[w][:], ath[:, epb, ibs], eth[:, epb, wsl],
                                    start=(epb == 0), stop=False,
                                )
                            for w in range(NJW):
                                wsl = slice(w * JW, (w + 1) * JW)
                                nc.tensor.matmul(
                                    pt_s[w][:], a8[:, epb, :, ibs],
                                    e8[:, epb, :, wsl],
                                    start=False, stop=False, perf_mode=DR,
                                )
                        # exact rank-2 part: R/2 into PSUM, closes the group
                        for w in range(NJW):
                            wsl = slice(w * JW, (w + 1) * JW)
                            nc.tensor.matmul(
                                pt_s[w][:], rki[:, ibs], rkj[:, wsl],
                                start=False, stop=True,
                            )
                        return pt_s

                    pt_s = emit_scores(0)
                    for ib in range(IB):
                        ibs = slice(ib * P, (ib + 1) * P)
                        # two-stage row max straight off PSUM
                        mx4 = p_sw.tile([P, NJW], f32, tag="mx4")
                        for w in range(NJW):
                            nc.vector.reduce_max(
                                mx4[:, w : w + 1], pt_s[w][:], axis=mybir.AxisListType.X
                            )
                        nmx = p_sw.tile([P, 1], f32, tag="nmx")
                        nc.vector.reduce_max(
                            nmx[:], mx4[:], axis=mybir.AxisListType.X, negate=True
                        )
                        nmx2 = p_sw.tile([P, 1], f32, tag="nmx2")
                        nc.vector.tensor_scalar_mul(nmx2[:], nmx[:], SCALE_EXP)
                        # unnormalized exp, fp16, straight off PSUM; normalization
                        # is deferred to the output evacuation (x 1/sum per i-row)
                        attn16 = p_sw.tile([P, S], f16, tag="attn16")
                        for w in range(NJW):
                            nc.scalar.activation(
                                attn16[:, w * JW : (w + 1) * JW], pt_s[w][:],
                                mybir.ActivationFunctionType.Exp,
                                bias=nmx2[:], scale=SCALE_EXP,
                            )
                        sm = p_sw.tile([P, 1], f32, tag="sm")
                        nc.vector.reduce_sum(sm[:], attn16[:], axis=mybir.AxisListType.X)
                        rs = p_sw.tile([P, 1], f32, tag="rs")
                        nc.vector.reciprocal(rs[:], sm[:])
                        if ib + 1 < IB:
                            pt_s = emit_scores(ib + 1)
                        # attn transpose on the (idle) DMA engines' XBAR path,
                        # freeing both the PE (tensor.transpose) and the DVE
                        # (PSUM->SBUF copies) it used to need
                        attnT = p_sw1.tile([P, JB, P], f16, tag="attnT")
                        for jb in range(JB):
                            nc.sync.dma_start_transpose(
                                attnT[:, jb, :], attn16[:, jb * P : (jb + 1) * P]
                            )
                        pt_o = [
                            ps.tile([P, OW], f32, tag="ps", name=f"ops_{ib}_{ob}")
                            for ob in range(NOW)
                        ]
                        for jb in range(JB):
                            for ob in range(NOW):
                                nc.tensor.matmul(
                                    pt_o[ob][:],
                                    attnT[:, jb, :],
                                    v16[:, jb, ob * OW : (ob + 1) * OW],
                                    start=(jb == 0), stop=(jb == JB - 1),
                                )
                        outt = p_sw1.tile([P, O], f32, tag="outt")
                        for ob in range(NOW):
                            osl = slice(ob * OW, (ob + 1) * OW)
                            nc.vector.tensor_scalar_mul(
                                outt[:, osl], pt_o[ob][:], rs[:]
                            )
                            nc.sync.dma_start(out[ibs, osl], outt[:, osl])

    nc.compile()
    return nc


_NC_CACHE = {}


def _get_nc(builder, *key):
    k = (builder.__name__,) + key
    if k not in _NC_CACHE:
        _NC_CACHE[k] = builder(*key)
    return _NC_CACHE[k]


def kernel(token_emb, W_q, W_k, W_v, mask=None, _trace=False, _tmpdir=None):
    token_emb = np.asarray(token_emb, np.float32)
    W_q = np.asarray(W_q, np.float32)
    W_k = np.asarray(W_k, np.float32)
    W_v = np.asarray(W_v, np.float32)
    B, S, E = token_emb.shape
    H = W_q.shape[0]
    O = W_v.shape[0]
    SI = S // 2
    EH = E // 2
    HQ = H // 4
    assert 2 * B == N_CORES

    # ---- launch 1: sharded Ghat = Wkc^T @ Wqc and V = emb @ W_v^T ----
    nc_g = _get_nc(build_g_nc, S, E, H, O)
    Wkc = W_k - 0.5
    Wqc = W_q - 0.5
    kh_f, kl_f = _split16(Wkc * 32.0)
    qh_f, ql_f = _split16(Wqc * 32.0)
    wvt = np.ascontiguousarray(W_v.T).astype(np.float16)
    emb_h = [
        _split16(np.ascontiguousarray(token_emb[b].T) * 32.0)[0] for b in range(B)
    ]
    g_maps = []
    for c in range(N_CORES):
        half, hq = c % 2, c // 2
        hsl = slice(hq * HQ, (hq + 1) * HQ)
        esl = slice(half * EH, (half + 1) * EH)
        b, jhalf = c // 2, c % 2
        k8 = np.empty((HQ, 2, E), F8)
        k8[:, 0, :] = _q8(kh_f[hsl].astype(np.float32), 1.0 / 16.0)
        k8[:, 1, :] = _q8(kl_f[hsl].astype(np.float32), 16.0)
        q8 = np.empty((HQ, 2, EH), F8)
        q8[:, 0, :] = _q8(ql_f[hsl, esl].astype(np.float32), 16.0)
        q8[:, 1, :] = _q8(qh_f[hsl, esl].astype(np.float32), 1.0 / 16.0)
        g_maps.append(
            {
                "kh16": np.ascontiguousarray(kh_f[hsl]),
                "k8": k8,
                "qh16": np.ascontiguousarray(qh_f[hsl, esl]),
                "q8": q8,
                "evt": np.ascontiguousarray(emb_h[b][:, jhalf * SI : (jhalf + 1) * SI]),
                "wvt": wvt,
            }
        )
    res_g = run_bass_kernel_spmd(
        nc_g, g_maps, core_ids=list(range(N_CORES)), trace=_trace,
        tmpdir=(_tmpdir + "/g" if _tmpdir else None),
    )
    Ghat = np.empty((E, E), np.float64)
    for half in range(2):
        esl = slice(half * EH, (half + 1) * EH)
        Ghat[:, esl] = sum(
            res_g.results[2 * q + half]["g_part"].astype(np.float64)
            for q in range(4)
        )
    g_h16, g_l16 = _split16((4.0 * Ghat).astype(np.float32))
    g_8 = np.empty((E, 2, E), F8)
    g_8[:, 0, :] = _q8(g_h16.astype(np.float32), 0.25)
    g_8[:, 1, :] = _q8(g_l16.astype(np.float32), 32.0)
    v_nat = [
        np.concatenate(
            [res_g.results[2 * b + 0]["v_part"], res_g.results[2 * b + 1]["v_part"]],
            axis=0,
        )
        for b in range(B)
    ]

    # rank-2 vectors (exact, host fp64): s = emb.1, p = emb.a, q = emb.b,
    # c = (H/4)s + p/2
    a_vec = Wqc.astype(np.float64).sum(0)
    b_vec = Wkc.astype(np.float64).sum(0)
    emb64 = token_emb.astype(np.float64)
    s_all = emb64.sum(2)                      # [B, S]
    p_all = emb64 @ a_vec                     # [B, S]
    q_all = emb64 @ b_vec                     # [B, S]
    c_all = (H / 4.0) * s_all + 0.5 * p_all

    # ---- launch 2: attention ----
    nc_main = _get_nc(build_main_nc, S, E, H, O)
    in_maps = []
    for c in range(N_CORES):
        b, half = divmod(c, 2)
        e = token_emb[b]
        hsl = slice(half * SI, (half + 1) * SI)
        osl = slice((1 - half) * SI, (2 - half) * SI)
        perm = np.concatenate([e[hsl], e[osl]], axis=0)
        et_h16, et_l16 = _split16(perm.T * 32.0)
        et_8 = np.empty((E, 2, S), F8)
        et_8[:, 0, :] = _q8(et_l16.astype(np.float32), 4.0)
        et_8[:, 1, :] = _q8(et_h16.astype(np.float32), 1.0 / 32.0)
        vp = v_nat[b]
        v_cat = np.concatenate([vp[hsl], vp[osl]], axis=0)
        s_p = np.concatenate([s_all[b][hsl], s_all[b][osl]])
        q_p = np.concatenate([q_all[b][hsl], q_all[b][osl]])
        c_p = np.concatenate([c_all[b][hsl], c_all[b][osl]])
        sih, sil = _split16((2.0 * s_p[:SI]).astype(np.float32))
        qih, qil = _split16((q_p[:SI] / 8.0).astype(np.float32))
        rk_i = np.ascontiguousarray(
            np.stack([sih, sih, sil, sil, qih, qih, qil, qil])
        )
        cjh, cjl = _split16((c_p / 4.0).astype(np.float32))
        sjh, sjl = _split16((2.0 * s_p).astype(np.float32))
        rk_j = np.ascontiguousarray(
            np.stack([cjh, cjl, cjh, cjl, sjh, sjl, sjh, sjl])
        )
        in_maps.append(
            {
                "g_h": g_h16, "g_8": g_8, "et_h": et_h16, "et_8": et_8,
                "v_in": np.ascontiguousarray(v_cat),
                "rk_i": rk_i, "rk_j": rk_j,
            }
        )
    res = run_bass_kernel_spmd(
        nc_main, in_maps, core_ids=list(range(N_CORES)), trace=_trace,
        tmpdir=(_tmpdir + "/main" if _tmpdir else None),
    )

    out = np.empty((B, S, O), np.float32)
    for c in range(N_CORES):
        b, half = divmod(c, 2)
        out[b, half * SI : (half + 1) * SI] = res.results[c]["out"]
    if _trace:
        kernel._last_results = (res_g, res)
    return out


# revision 8
# speedup vs baseline: 1.0579x; 1.0579x over previous
"""CavemanGPT single-head attention on 8 Trainium2 NeuronCores.

Math (reference; its mask input is unused there):
    Q = emb @ W_q^T ; K = emb @ W_k^T ; V = emb @ W_v^T        (per batch b)
    out = softmax(K @ Q^T / sqrt(H), axis=-1) @ V

Algebraic restructure 1 (from the baseline): K @ Q^T = emb @ (W_k^T W_q) @ emb^T,
so with G := W_k^T @ W_q ([E, E], batch independent) the per-core work drops
~3.2x and the giant [S, H] Q/K intermediates vanish.

Restructure 2 (new): W_q/W_k are uniform[0,1) = 0.5 + centered part, so
    G = 0.25*H*1 1^T + 0.5*1 a^T + 0.5*b 1^T + Ghat,   Ghat = Wkc^T @ Wqc
with a = Wqc^T 1, b = Wkc^T 1.  The rank-2 part carries ~99% of the score
magnitude (|scores| to 1.1e7) and is computed EXACTLY via an 8-row fp16
matmul from host-precomputed vectors (s_i = sum(emb_i), c_j = (H/4)s_j +
p_j/2, q_i = b.emb_i):
    scores = s_i c_j + q_i s_j/2 + emb_i Ghat emb_j^T
The residual Ghat-scores are ~180x smaller (std ~8e3), so the fp32-grade
hi/lo-limb chain of the baseline is overkill for them.  Each residual
product x*y with fp16 limbs (xh+xl)(yh+yl) is computed as ONE fp16 matmul
(xh*yh) plus ONE fp8e4 DoubleRow matmul computing BOTH cross terms
(xh*yl + xl*yh) in a single pass at 2x rate -- ~2/3 the PE time of the
baseline's 3x fp16 limb scheme with BETTER accuracy (the cross terms only
need ~5 bits below the hi product; e4m3 quantization of the limbs leaves
~2^-15.5-grade products, vs needing ~14.5 bits for the residual).

Softmax here is a hard argmax (top-2 score gaps median ~870 in exp-arg
units, min 0.21): correctness = never flip an argmax; verified exactly in
simulation on the real inputs (0 flips, max rel err 8.1e-4, identical to
the full 3-limb fp16 scheme -- the fp16 attn/V floor).

Two launches:
  1. G-launch: Ghat = Wkc^T @ Wqc sharded over 8 cores (2 e'-halves x 4
     h-quarters), fp16 hh + fp8 DoubleRow cross; host sums the h-partials.
     Plus one (batch, j-half) shard of V = emb @ W_v^T per core (fp16).
  2. Main launch: 8 cores = 4 batches x 2 halves of the i (output-row)
     dimension, each producing out[i-half] via residual scores + rank-2
     matmul + softmax + attn @ V.
"""

import math

import numpy as np
import ml_dtypes

import concourse.bass as bass
import concourse.bass_utils as _bu
import concourse.mybir as mybir
import concourse.tile as tile
from concourse import bacc
from concourse.bass_utils import run_bass_kernel_spmd
from concourse.masks import make_identity

# LDWEIGHTS dedup: consecutive matmuls sharing a stationary operand skip the
# reload. Verified to produce bit-identical output on this kernel.
if not getattr(_bu, "_ldw_opt_patched", False):
    _orig_walrus_args = _bu.get_walrus_args

    def _walrus_args_ldw(arch, tmpdir, *, dve_root=None):
        args = _orig_walrus_args(arch, tmpdir, dve_root=dve_root)
        return [a.replace("--enable-ldw-opt=false", "--enable-ldw-opt=true") for a in args]

    _bu.get_walrus_args = _walrus_args_ldw
    _bu._ldw_opt_patched = True

dt = mybir.dt
F8 = ml_dtypes.float8_e4m3
P = 128
N_CORES = 8
DR = mybir.MatmulPerfMode.DoubleRow


def _split16(x):
    """x (fp32) -> (hi, lo) fp16 limbs with x ~= hi + lo (22-bit mantissa)."""
    x = np.ascontiguousarray(x, dtype=np.float32)
    hi = x.astype(np.float16)
    lo = (x - hi.astype(np.float32)).astype(np.float16)
    return hi, lo


def _q8(x, scale):
    """fp32 -> TRN e4m3 of x*scale (saturating clip to +-240)."""
    y = np.clip(np.asarray(x, np.float32) * scale, -240.0, 240.0)
    return np.ascontiguousarray(y).astype(F8)


def build_g_nc(S, E, H, O):
    """Launch 1: per-core partial Ghat' = Wkc[hq]^T @ Wqc[hq][:, e'half]
    (PSUM = (32Wkc)^T(32Wqc) = 1024*Ghat) plus one (batch, j-half) shard of
    V = embT^T @ WvT (single fp16).

    Limb scheme per 128-h chunk: 1 fp16 matmul (Kh^T Qh) + 1 fp8e4 DoubleRow
    matmul computing Kh^T Ql + Kl^T Qh (stored pre-scaled so both products
    land in the same PSUM units).
    """
    SI = S // 2
    EH = E // 2
    HQ = H // 4
    EB = E // P
    HCB = HQ // P
    JBH = SI // P
    GW = min(512, EH)
    OW = min(512, O)
    NOW = O // OW
    f32, f16, f8 = dt.float32, dt.float16, dt.float8e4

    nc = bacc.Bacc("TRN2", target_bir_lowering=False, debug=False)
    kh16 = nc.dram_tensor("kh16", [HQ, E], f16, kind="ExternalInput").ap()
    k8 = nc.dram_tensor("k8", [HQ, 2, E], f8, kind="ExternalInput").ap()
    qh16 = nc.dram_tensor("qh16", [HQ, EH], f16, kind="ExternalInput").ap()
    q8 = nc.dram_tensor("q8", [HQ, 2, EH], f8, kind="ExternalInput").ap()
    evt = nc.dram_tensor("evt", [E, SI], f16, kind="ExternalInput").ap()
    wvt = nc.dram_tensor("wvt", [E, O], f16, kind="ExternalInput").ap()
    g_part = nc.dram_tensor("g_part", [E, EH], f32, kind="ExternalOutput").ap()
    v_part = nc.dram_tensor("v_part", [SI, O], f16, kind="ExternalOutput").ap()

    with tile.TileContext(nc) as tc:
        with (
            tc.tile_pool(name="p_res", bufs=1) as p_res,
            tc.tile_pool(name="p_vo", bufs=2) as p_vo,
            tc.tile_pool(name="p_gs", bufs=3) as p_gs,
            tc.tile_pool(name="ps_g", bufs=8, space="PSUM") as ps_g,
        ):
            # ---- PE warm-up: ~3.5us of dummy matmuls during the DMA
            # preamble trips the HAM clock-gate so real matmuls start at
            # 2.4GHz instead of 1.2 ----
            wu = p_res.tile([P, P], f16)
            nc.gpsimd.memset(wu[:], 0.0)
            wups = ps_g.tile([P, P], f32, tag="gps", name="wups")
            for _ in range(48):
                nc.tensor.matmul(wups[:], wu[:], wu[:], start=True, stop=True)

            # ---- Ghat partial ----
            gp = p_res.tile([P, EB, EH], f32)
            evc = p_res.tile([P, EB, SI], f16)
            wvc = p_res.tile([P, EB, O], f16)
            evr = evt.rearrange("(eo p) j -> p eo j", p=P)
            wvr = wvt.rearrange("(eo p) o -> p eo o", p=P)
            pt_g = [
                ps_g.tile([P, GW], f32, tag="gps", name=f"gps_{eb}")
                for eb in range(EB)
            ]
            for hc in range(HCB):
                hs = slice(hc * P, (hc + 1) * P)
                kh = p_gs.tile([P, E], f16, tag="kh")
                nc.sync.dma_start(kh[:], kh16[hs, :])
                qh = p_gs.tile([P, EH], f16, tag="qh")
                nc.sync.dma_start(qh[:], qh16[hs, :])
                k8t = p_gs.tile([P, 2, E], f8, tag="k8")
                nc.sync.dma_start(k8t[:], k8[hs])
                q8t = p_gs.tile([P, 2, EH], f8, tag="q8")
                nc.sync.dma_start(q8t[:], q8[hs])
                # stream the V inputs alongside the k/q chunks so the V
                # matmuls can start the moment the Ghat ones finish
                nc.sync.dma_start(evc[:, hc], evr[:, hc])
                nc.sync.dma_start(wvc[:, hc], wvr[:, hc])
                first, last = hc == 0, hc == HCB - 1
                for eb in range(EB):
                    ksl = slice(eb * P, (eb + 1) * P)
                    nc.tensor.matmul(
                        pt_g[eb][:], kh[:, ksl], qh[:], start=first, stop=False,
                    )
                    nc.tensor.matmul(
                        pt_g[eb][:], k8t[:, :, ksl], q8t[:],
                        start=False, stop=last, perf_mode=DR,
                    )
            gpr = g_part.rearrange("(eo p) e2 -> p eo e2", p=P)
            for eb in range(EB):
                nc.vector.tensor_scalar_mul(gp[:, eb], pt_g[eb][:], 2.0**-10)
                # overlap the writeback with the remaining evacuations
                nc.sync.dma_start(gpr[:, eb], gp[:, eb])

            # ---- V shard (PE runs it after Ghat; inputs loaded during it) ----
            for jb in range(JBH):
                jsl = slice(jb * P, (jb + 1) * P)
                pv_tiles = [
                    ps_g.tile([P, OW], f32, tag="gps", name=f"vps_{jb}_{ob}")
                    for ob in range(NOW)
                ]
                for eb in range(EB):
                    for ob in range(NOW):
                        osl = slice(ob * OW, (ob + 1) * OW)
                        nc.tensor.matmul(
                            pv_tiles[ob][:], evc[:, eb, jsl], wvc[:, eb, osl],
                            start=(eb == 0), stop=(eb == EB - 1),
                        )
                vt = p_vo.tile([P, O], f16, tag="vt")
                for ob in range(NOW):
                    osl = slice(ob * OW, (ob + 1) * OW)
                    nc.vector.tensor_scalar_mul(vt[:, osl], pv_tiles[ob][:], 2.0**-5)
                    nc.sync.dma_start(v_part[jsl, osl], vt[:, osl])

    nc.compile()
    return nc


def build_main_nc(S, E, H, O):
    """Launch 2: attention for one (batch, i-half).

    Residual chain (PSUM units: stage1 = 128*Ghat@emb, stage2 = raw/2):
      AT = Ghat^T @ embT:  Gh16*Eh (fp16) + DoubleRow[gh8*el8 + gl8*eh8]
      scores: Ah*Eh (fp16) + DoubleRow[ah8*el8 + al8*eh8] + 8-row rank matmul
    Stored forms: Eh = fp16(32 emb^T), e8 = [e4m3(4 El) | e4m3(Eh/32)],
    Gh/Gl = fp16 pair of 4*Ghat, g8 = [e4m3(Gh/4) | e4m3(32 Gl)],
    Ah = fp16(AT*2^-6) (psum*2^-13), a8 = [e4m3(Ah/4) | e4m3(32 Al)].
    Rank matmul rows (i-side | j-side): fp16 limb pairs of
    (2s_i, q_i/8 | c_j/4, 2s_j) arranged so their PSUM sum is R_ij/2.
    """
    SI = S // 2          # i rows per core
    EB = E // P          # 128-chunks of the embedding dim
    JB = S // P
    IB = SI // P
    IW = min(512, SI)    # AT moving width along i
    NIH = SI // IW
    JW = min(512, S)     # scores moving width along j
    NJW = S // JW
    OW = min(512, O)
    NOW = O // OW
    SCALE_EXP = 2.0 / math.sqrt(H)   # PSUM = raw/2

    f32, f16, f8 = dt.float32, dt.float16, dt.float8e4

    nc = bacc.Bacc("TRN2", target_bir_lowering=False, debug=False)
    g_h = nc.dram_tensor("g_h", [E, E], f16, kind="ExternalInput").ap()
    g_8 = nc.dram_tensor("g_8", [E, 2, E], f8, kind="ExternalInput").ap()
    et_h = nc.dram_tensor("et_h", [E, S], f16, kind="ExternalInput").ap()
    et_8 = nc.dram_tensor("et_8", [E, 2, S], f8, kind="ExternalInput").ap()
    v_in = nc.dram_tensor("v_in", [S, O], f16, kind="ExternalInput").ap()
    rk_i = nc.dram_tensor("rk_i", [8, SI], f16, kind="ExternalInput").ap()
    rk_j = nc.dram_tensor("rk_j", [8, S], f16, kind="ExternalInput").ap()
    out = nc.dram_tensor("out", [SI, O], f32, kind="ExternalOutput").ap()

    with tile.TileContext(nc) as tc:
        with (
            tc.tile_pool(name="misc", bufs=2) as misc,
            tc.tile_pool(name="p_big", bufs=1) as p_big,
        ):
            ident = misc.tile([P, P], f16, tag="ident", name="ident")
            make_identity(nc, ident[:])
            wu = misc.tile([P, P], f16, tag="wu", name="wu")
            nc.gpsimd.memset(wu[:], 0.0)

            # whole-kernel residents
            eth = p_big.tile([P, EB, S], f16)     # Eh: [e part, e chunk, tok]
            e8 = p_big.tile([P, EB, 2, S], f8)    # [4El | Eh/32]
            ath = p_big.tile([P, EB, SI], f16)    # Ah: [e' part, e' chunk, i]
            a8 = p_big.tile([P, EB, 2, SI], f8)   # [Ah/4 | 32Al]
            v16 = p_big.tile([P, JB, O], f16)     # V: [j part, j chunk, o]
            rki = p_big.tile([8, SI], f16)
            rkj = p_big.tile([8, S], f16)

            with tc.tile_pool(name="ps", bufs=8, space="PSUM") as ps:
                # PE warm-up during the input-DMA preamble (see launch 1)
                wups = ps.tile([P, P], f32, tag="ps", name="wups")
                for _ in range(48):
                    nc.tensor.matmul(wups[:], wu[:], wu[:], start=True, stop=True)

                # ---- AT = Ghat^T embT (PSUM = 128*Ghat@emb -> *2^-13) ----
                with tc.tile_pool(name="p_g", bufs=1) as p_g:
                    gh = p_g.tile([P, EB, E], f16)
                    g8t = p_g.tile([P, EB, 2, E], f8)
                    # DMAs emitted in first-use order, chunked per e-block so
                    # the first AT matmuls start early.
                    ghr = g_h.rearrange("(eo p) e2 -> p eo e2", p=P)
                    g8r = g_8.rearrange("(eo p) two e2 -> p eo two e2", p=P)
                    ethr = et_h.rearrange("(eo p) t -> p eo t", p=P)
                    e8r = et_8.rearrange("(eo p) two t -> p eo two t", p=P)
                    for eb in range(EB):
                        nc.sync.dma_start(gh[:, eb], ghr[:, eb])
                        nc.sync.dma_start(eth[:, eb, :SI], ethr[:, eb, :SI])
                        nc.sync.dma_start(g8t[:, eb], g8r[:, eb])
                        nc.sync.dma_start(e8[:, eb, :, :SI], e8r[:, eb, :, :SI])
                    if SI < S:
                        nc.sync.dma_start(eth[:, :, SI:], ethr[:, :, SI:])
                        for eb in range(EB):
                            nc.sync.dma_start(
                                e8[:, eb, :, SI:], e8r[:, eb, :, SI:]
                            )
                    nc.sync.dma_start(rkj[:], rk_j)
                    nc.sync.dma_start(rki[:], rk_i)
                    nc.sync.dma_start(
                        v16[:], v_in.rearrange("(jo p) o -> p jo o", p=P)
                    )
                    for ih in range(NIH):
                        isl = slice(ih * IW, (ih + 1) * IW)
                        pts = [
                            ps.tile([P, IW], f32, tag="ps", name=f"aps_{ih}_{epb}")
                            for epb in range(EB)
                        ]
                        for eb in range(EB):
                            first, last = eb == 0, eb == EB - 1
                            for epb in range(EB):
                                psl = slice(epb * P, (epb + 1) * P)
                                nc.tensor.matmul(
                                    pts[epb][:], gh[:, eb, psl], eth[:, eb, isl],
                                    start=first, stop=False,
                                )
                                nc.tensor.matmul(
                                    pts[epb][:], g8t[:, eb, :, psl],
                                    e8[:, eb, :, isl],
                                    start=False, stop=last, perf_mode=DR,
                                )
                        for epb in range(EB):
                            pt = pts[epb]
                            atmp = misc.tile([P, IW], f32, tag="atmp", name=f"atmp_{ih}_{epb}")
                            alo = misc.tile([P, IW], f32, tag="alo", name=f"alo_{ih}_{epb}")
                            nc.vector.tensor_scalar_mul(atmp[:], pt[:], 2.0**-13)
                            nc.vector.tensor_copy(ath[:, epb, isl], atmp[:])
                            nc.vector.tensor_tensor(
                                alo[:], atmp[:], ath[:, epb, isl],
                                mybir.AluOpType.subtract,
                            )
                            # fp8 limb casts on the (idle) scalar engine
                            nc.scalar.activation(
                                a8[:, epb, 0, isl], ath[:, epb, isl],
                                mybir.ActivationFunctionType.Copy, scale=0.25,
                            )
                            nc.scalar.activation(
                                a8[:, epb, 1, isl], alo[:],
                                mybir.ActivationFunctionType.Copy, scale=32.0,
                            )

                # ---- scores + softmax + out, fused per 128-row i block ----
                with (
                    tc.tile_pool(name="p_sw", bufs=2) as p_sw,
                    tc.tile_pool(name="p_sw1", bufs=2) as p_sw1,
                ):
                    def emit_scores(ib):
                        ibs = slice(ib * P, (ib + 1) * P)
                        pt_s = [
                            ps.tile([P, JW], f32, tag="ps", name=f"sps_{ib}_{w}")
                            for w in range(NJW)
                        ]
                        for epb in range(EB):
                            for w in range(NJW):
                                wsl = slice(w * JW, (w + 1) * JW)
                                nc.tensor.matmul(
                                    pt_s[w][:], ath[:, epb, ibs], eth[:, epb, wsl],
                                    start=(epb == 0), stop=False,
                                )
                            for w in range(NJW):
                                wsl = slice(w * JW, (w + 1) * JW)
                                nc.tensor.matmul(
                                    pt_s[w][:], a8[:, epb, :, ibs],
                                    e8[:, epb, :, wsl],
                                    start=False, stop=False, perf_mode=DR,
                                )
                        # exact rank-2 part: R/2 into PSUM, closes the group
                        for w in range(NJW):
                            wsl = slice(w * JW, (w + 1) * JW)
                            nc.tensor.matmul(
                                pt_s[w][:], rki[:, ibs], rkj[:, wsl],
                                start=False, stop=True,
                            )
                        return pt_s

                    pt_s = emit_scores(0)
                    for ib in range(IB):
                        ibs = slice(ib * P, (ib + 1) * P)
                        # two-stage row max straight off PSUM
                        mx4 = p_sw.tile([P, NJW], f32, tag="mx4")
                        for w in range(NJW):
                            nc.vector.reduce_max(
                                mx4[:, w : w + 1], pt_s[w][:], axis=mybir.AxisListType.X
                            )
                        nmx = p_sw.tile([P, 1], f32, tag="nmx")
                        nc.vector.reduce_max(
                            nmx[:], mx4[:], axis=mybir.AxisListType.X, negate=True
                        )
                        nmx2 = p_sw.tile([P, 1], f32, tag="nmx2")
                        nc.vector.tensor_scalar_mul(nmx2[:], nmx[:], SCALE_EXP)
                        # unnormalized exp, fp16, straight off PSUM; normalization
                        # is deferred to the output evacuation (x 1/sum per i-row)
                        attn16 = p_sw.tile([P, S], f16, tag="attn16")
                        for w in range(NJW):
                            nc.scalar.activation(
                                attn16[:, w * JW : (w + 1) * JW], pt_s[w][:],
                                mybir.ActivationFunctionType.Exp,
                                bias=nmx2[:], scale=SCALE_EXP,
                            )
                        sm = p_sw.tile([P, 1], f32, tag="sm")
                        nc.vector.reduce_sum(sm[:], attn16[:], axis=mybir.AxisListType.X)
                        rs = p_sw.tile([P, 1], f32, tag="rs")
                        nc.vector.reciprocal(rs[:], sm[:])
                        if ib + 1 < IB:
                            pt_s = emit_scores(ib + 1)
                        attnT = p_sw1.tile([P, JB, P], f16, tag="attnT")
                        for jb in range(JB):
                            tp = ps.tile([P, P], f16, tag="ps", name=f"tps_{ib}_{jb}")
                            nc.tensor.transpose(
                                tp[:], attn16[:, jb * P : (jb + 1) * P], ident[:]
                            )
                            nc.vector.tensor_copy(attnT[:, jb, :], tp[:])
                        pt_o = [
                            ps.tile([P, OW], f32, tag="ps", name=f"ops_{ib}_{ob}")
                            for ob in range(NOW)
                        ]
                        for jb in range(JB):
                            for ob in range(NOW):
                                nc.tensor.matmul(
                                    pt_o[ob][:],
                                    attnT[:, jb, :],
                                    v16[:, jb, ob * OW : (ob + 1) * OW],
                                    start=(jb == 0), stop=(jb == JB - 1),
                                )
                        outt = p_sw1.tile([P, O], f32, tag="outt")
                        for ob in range(NOW):
                            osl = slice(ob * OW, (ob + 1) * OW)
                            nc.vector.tensor_scalar_mul(
                                outt[:, osl], pt_o[ob][:], rs[:]
                            )
                            nc.sync.dma_start(out[ibs, osl], outt[:, osl])

    nc.compile()
    return nc


_NC_CACHE = {}


def _get_nc(builder, *key):
    k = (builder.__name__,) + key
    if k not in _NC_CACHE:
        _NC_CACHE[k] = builder(*key)
    return _NC_CACHE[k]


def kernel(token_emb, W_q, W_k, W_v, mask=None, _trace=False, _tmpdir=None):
    token_emb = np.asarray(token_emb, np.float32)
    W_q = np.asarray(W_q, np.float32)
    W_k = np.asarray(W_k, np.float32)
    W_v = np.asarray(W_v, np.float32)
    B, S, E = token_emb.shape
    H = W_q.shape[0]
    O = W_v.shape[0]
    SI = S // 2
    EH = E // 2
    HQ = H // 4
    assert 2 * B == N_CORES

    # ---- launch 1: sharded Ghat = Wkc^T @ Wqc and V = emb @ W_v^T ----
    nc_g = _get_nc(build_g_nc, S, E, H, O)
    Wkc = W_k - 0.5
    Wqc = W_q - 0.5
    kh_f, kl_f = _split16(Wkc * 32.0)
    qh_f, ql_f = _split16(Wqc * 32.0)
    wvt = np.ascontiguousarray(W_v.T).astype(np.float16)
    emb_h = [
        _split16(np.ascontiguousarray(token_emb[b].T) * 32.0)[0] for b in range(B)
    ]
    g_maps = []
    for c in range(N_CORES):
        half, hq = c % 2, c // 2
        hsl = slice(hq * HQ, (hq + 1) * HQ)
        esl = slice(half * EH, (half + 1) * EH)
        b, jhalf = c // 2, c % 2
        k8 = np.empty((HQ, 2, E), F8)
        k8[:, 0, :] = _q8(kh_f[hsl].astype(np.float32), 1.0 / 16.0)
        k8[:, 1, :] = _q8(kl_f[hsl].astype(np.float32), 16.0)
        q8 = np.empty((HQ, 2, EH), F8)
        q8[:, 0, :] = _q8(ql_f[hsl, esl].astype(np.float32), 16.0)
        q8[:, 1, :] = _q8(qh_f[hsl, esl].astype(np.float32), 1.0 / 16.0)
        g_maps.append(
            {
                "kh16": np.ascontiguousarray(kh_f[hsl]),
                "k8": k8,
                "qh16": np.ascontiguousarray(qh_f[hsl, esl]),
                "q8": q8,
                "evt": np.ascontiguousarray(emb_h[b][:, jhalf * SI : (jhalf + 1) * SI]),
                "wvt": wvt,
            }
        )
    res_g = run_bass_kernel_spmd(
        nc_g, g_maps, core_ids=list(range(N_CORES)), trace=_trace,
        tmpdir=(_tmpdir + "/g" if _tmpdir else None),
    )
    Ghat = np.empty((E, E), np.float64)
    for half in range(2):
        esl = slice(half * EH, (half + 1) * EH)
        Ghat[:, esl] = sum(
            res_g.results[2 * q + half]["g_part"].astype(np.float64)
            for q in range(4)
        )
    g_h16, g_l16 = _split16((4.0 * Ghat).astype(np.float32))
    g_8 = np.empty((E, 2, E), F8)
    g_8[:, 0, :] = _q8(g_h16.astype(np.float32), 0.25)
    g_8[:, 1, :] = _q8(g_l16.astype(np.float32), 32.0)
    v_nat = [
        np.concatenate(
            [res_g.results[2 * b + 0]["v_part"], res_g.results[2 * b + 1]["v_part"]],
            axis=0,
        )
        for b in range(B)
    ]

    # rank-2 vectors (exact, host fp64): s = emb.1, p = emb.a, q = emb.b,
    # c = (H/4)s + p/2
    a_vec = Wqc.astype(np.float64).sum(0)
    b_vec = Wkc.astype(np.float64).sum(0)
    emb64 = token_emb.astype(np.float64)
    s_all = emb64.sum(2)                      # [B, S]
    p_all = emb64 @ a_vec                     # [B, S]
    q_all = emb64 @ b_vec                     # [B, S]
    c_all = (H / 4.0) * s_all + 0.5 * p_all

    # ---- launch 2: attention ----
    nc_main = _get_nc(build_main_nc, S, E, H, O)
    in_maps = []
    for c in range(N_CORES):
        b, half = divmod(c, 2)
        e = token_emb[b]
        hsl = slice(half * SI, (half + 1) * SI)
        osl = slice((1 - half) * SI, (2 - half) * SI)
        perm = np.concatenate([e[hsl], e[osl]], axis=0)
        et_h16, et_l16 = _split16(perm.T * 32.0)
        et_8 = np.empty((E, 2, S), F8)
        et_8[:, 0, :] = _q8(et_l16.astype(np.float32), 4.0)
        et_8[:, 1, :] = _q8(et_h16.astype(np.float32), 1.0 / 32.0)
        vp = v_nat[b]
        v_cat = np.concatenate([vp[hsl], vp[osl]], axis=0)
        s_p = np.concatenate([s_all[b][hsl], s_all[b][osl]])
        q_p = np.concatenate([q_all[b][hsl], q_all[b][osl]])
        c_p = np.concatenate([c_all[b][hsl], c_all[b][osl]])
        sih, sil = _split16((2.0 * s_p[:SI]).astype(np.float32))
        qih, qil = _split16((q_p[:SI] / 8.0).astype(np.float32))
        rk_i = np.ascontiguousarray(
            np.stack([sih, sih, sil, sil, qih, qih, qil, qil])
        )
        cjh, cjl = _split16((c_p / 4.0).astype(np.float32))
        sjh, sjl = _split16((2.0 * s_p).astype(np.float32))
        rk_j = np.ascontiguousarray(
            np.stack([cjh, cjl, cjh, cjl, sjh, sjl, sjh, sjl])
        )
        in_maps.append(
            {
                "g_h": g_h16, "g_8": g_8, "et_h": et_h16, "et_8": et_8,
                "v_in": np.ascontiguousarray(v_cat),
                "rk_i": rk_i, "rk_j": rk_j,
            }
        )
    res = run_bass_kernel_spmd(
        nc_main, in_maps, core_ids=list(range(N_CORES)), trace=_trace,
        tmpdir=(_tmpdir + "/main" if _tmpdir else None),
    )

    out = np.empty((B, S, O), np.float32)
    for c in range(N_CORES):
        b, half = divmod(c, 2)
        out[b, half * SI : (half + 1) * SI] = res.results[c]["out"]
    if _trace:
        kernel._last_results = (res_g, res)
    return out
